# revision 1
# baseline (speedup 1.0000x reference)
"""Multi-head attention with learned memory slots, 8-way sharded for TRN2.

Sharding: 8 cores = 4 batches x 2 head-groups.
  core c -> batch b = c//2, head group g = c%2 (heads 8g..8g+7).
  Wq/Wk/Wv column-sharded by head group, mk/mv sharded on h*d axis,
  Wo row-sharded; pairwise ReduceScatter(add) combines the two head
  groups of a batch and scatters the query rows, so the host only
  concatenates slices.

Device kernel (identical SPMD program, per-core data differs):
  - inputs loaded in one DMA each, transposed on PE to [D, S] layout
  - projections produce Q^T/K^T [hd, seq] and V [seq, hd] directly
  - per head: scores^T = K_h^T.T @ Q_h^T -> exp on ACT (no max
    subtraction needed: |scores| <~ 8, exp is safe in fp32)
  - AV with a ones-column appended to V gives the softmax denominator
    in the same accumulation for free (out rows 0..63, sums row 64)
  - normalization: in-place reciprocal of the sums row, GpSimd
    partition_broadcast, one elementwise multiply per head
  - output projection contracts in K=64 tiles (outT stays at base
    partition 0), then pairwise ReduceScatter
  - matmuls run as float32r (full-rate fp32 mode for moving dim >= 256)
"""

import math
import os
from contextlib import ExitStack

import numpy as np

import concourse.bass as bass
import concourse.mybir as mybir
import concourse.tile as tile
from concourse import bacc
from concourse.bass_utils import run_bass_kernel_spmd
from concourse.masks import make_identity

F32 = mybir.dt.float32
MM_DT = mybir.dt.float32r  # matmul operand view; float32r = fast fp32

B = 4
S = 1024          # sequence length (also #queries)
D = 1024          # model dim
NH = 8            # heads per core
DK = 64           # head dim
HD = NH * DK      # 512, per-core head*dim
M = 128           # memory slots
SKM = S + M       # 1152 keys incl. memory slots
NKC = SKM // 128  # 9 key chunks
UNITS = 1024
SCALE_M = math.sqrt(float(M))
INV_SQRT_DK = 1.0 / math.sqrt(float(DK))

# key-chunk groups per exp tile: (0,1),(2,3),(4,5),(6,7),(8,)
KC_GROUPS = [(0, 1), (2, 3), (4, 5), (6, 7), (8,)]

_CACHED = {}


def _mm(ap):
    return ap.bitcast(MM_DT)


def _bcast_ap(ap, nparts):
    """Partition-broadcast AP: same free pattern on nparts partitions."""
    return bass.AP(tensor=ap.tensor, offset=ap.offset, ap=[[0, nparts]] + list(ap.ap))


def build_nc(es_pair=False, evac_engine="scalar", sc_bufs=2, es_bufs=3,
             stop_after="full"):
    nc = bacc.Bacc("TRN2", target_bir_lowering=False, debug=False, num_devices=8)
    kc_groups = KC_GROUPS if es_pair else [(kc,) for kc in range(NKC)]

    xq_e = nc.dram_tensor("xq", [S, D], F32, kind="ExternalInput")
    xk_e = nc.dram_tensor("xk", [S, D], F32, kind="ExternalInput")
    xv_e = nc.dram_tensor("xv", [S, D], F32, kind="ExternalInput")
    wq_e = nc.dram_tensor("wq", [D, HD], F32, kind="ExternalInput")
    wk_e = nc.dram_tensor("wk", [D, HD], F32, kind="ExternalInput")
    wv_e = nc.dram_tensor("wv", [D, HD], F32, kind="ExternalInput")
    bq_e = nc.dram_tensor("bq", [HD], F32, kind="ExternalInput")
    bk_e = nc.dram_tensor("bk", [HD], F32, kind="ExternalInput")
    bv_e = nc.dram_tensor("bv", [HD], F32, kind="ExternalInput")
    wo_e = nc.dram_tensor("wo", [HD, UNITS], F32, kind="ExternalInput")
    bo_e = nc.dram_tensor("bo", [UNITS], F32, kind="ExternalInput")
    mk_e = nc.dram_tensor("mk", [M, HD], F32, kind="ExternalInput")
    mv_e = nc.dram_tensor("mv", [M, HD], F32, kind="ExternalInput")
    out_e = nc.dram_tensor("out", [S // 2, UNITS], F32, kind="ExternalOutput")

    with tile.TileContext(nc) as tc, ExitStack() as ctx:
        consts = ctx.enter_context(tc.tile_pool(name="consts", bufs=1))
        dram = ctx.enter_context(tc.tile_pool(name="dram", bufs=1, space="DRAM"))

        identity = consts.tile([128, 128], F32)
        make_identity(nc, identity)

        # biases: bq/bk as [128, 4] per-partition scalars (hd on partitions)
        bq_t = consts.tile([128, 4], F32)
        bk_t = consts.tile([128, 4], F32)
        nc.gpsimd.dma_start(out=bq_t, in_=bq_e[:].rearrange("(mt p) -> p mt", p=128))
        nc.gpsimd.dma_start(out=bk_t, in_=bk_e[:].rearrange("(mt p) -> p mt", p=128))
        # bv/bo broadcast along partitions (they index the free dim)
        bv_bc = consts.tile([128, HD], F32)
        bo_bc = consts.tile([128, UNITS], F32)
        nc.gpsimd.dma_start(out=bv_bc, in_=_bcast_ap(bv_e[:], 128))
        nc.gpsimd.dma_start(out=bo_bc, in_=_bcast_ap(bo_e[:], 128))

        mk_sb = consts.tile([M, HD], F32)
        mv_sb = consts.tile([M, HD], F32)
        nc.sync.dma_start(out=mk_sb, in_=mk_e[:])
        nc.sync.dma_start(out=mv_sb, in_=mv_e[:])

        # Wo loaded early so the DMA overlaps earlier phases; SWDGE casts
        # fp32 -> fp32r in flight (matmul operands must be f32r-rounded)
        wo_sb = consts.tile([64, NH, UNITS], F32)
        nc.gpsimd.dma_start(
            out=_mm(wo_sb[:]), in_=wo_e[:].rearrange("(h p) c -> p h c", p=64)
        )

        partial = dram.tile([S, UNITS], F32)
        rs_out = dram.tile([S // 2, UNITS], F32)

        with tc.tile_pool(name="qkv", bufs=1) as qkv_pool:
            qT = qkv_pool.tile([128, 4, S], F32)      # [hd_low, hd_grp, q]
            kT = qkv_pool.tile([128, 4, SKM], F32)    # [hd_low, hd_grp, k]
            vt = qkv_pool.tile([128, NKC, NH * 66], F32)  # [k_low, k_chunk, h*66]

            # V layout: head block h = 66 cols: [V_h(64) | ones | pad-ones]
            # (f32r writes/reads need 8B alignment and even element counts;
            #  memset can't emit f32r, so a SWDGE cast-DMA scatters the ones)
            ones_col = consts.tile([128, 2], F32)
            nc.vector.memset(ones_col, 1.0)
            oc = ones_col[:]
            # [1, 64] f32r ones row AT PARTITION 64: lhsT for the K=1
            # recip-broadcast matmuls (matmul lhsT/rhs must share their
            # base partition, and the sums live on partition 64)
            ones_t = consts.tile([65, 64], F32)
            nc.gpsimd.dma_start(
                out=_mm(ones_t[64:65, 0:64]),
                in_=bass.AP(tensor=oc.tensor, offset=oc.offset,
                            ap=[[oc.ap[0][0], 1], [0, 32], [1, 2]]),
            )
            ones_src = bass.AP(
                tensor=oc.tensor, offset=oc.offset,
                ap=[list(oc.ap[0]), [0, NKC * NH], [1, 2]],
            )
            nc.gpsimd.dma_start(
                out=_mm(vt[:].rearrange("p kc (b c) -> p (kc b) c", c=66)[:, :, 64:66]),
                in_=ones_src,
            )
            # memory-slot rows of V (k chunk 8): scale_m * mv, no bias
            nc.vector.tensor_scalar_mul(
                _mm(vt[:, NKC - 1, :].rearrange("p (h c) -> p h c", c=66)[:, :, 0:64]),
                mv_sb[:].rearrange("p (h c) -> p h c", c=64),
                SCALE_M,
            )

            # ---- input transpose + projections ---------------------------
            with tc.tile_pool(name="wproj", bufs=1) as wpool, \
                 tc.tile_pool(name="slab", bufs=1) as slab_pool, \
                 tc.tile_pool(name="xT", bufs=1) as xT_pool, \
                 tc.tile_pool(name="tr_ps", bufs=2, space="PSUM") as tr_pool, \
                 tc.tile_pool(name="proj_ps", bufs=2, space="PSUM") as proj_pool:

                def transpose_input(x_ext):
                    """DRAM [S, D] -> SBUF x^T [128, 8, S] ([d_low, dc, s])."""
                    slab = slab_pool.tile([128, 8, D], F32, tag="slab")
                    # two DMAs: slab[p, sc, c] = x[sc*128 + p, c]; the split
                    # lets the first transposes start at the halfway point
                    x_r = x_ext[:].rearrange("(sc p) c -> p sc c", p=128)
                    nc.sync.dma_start(out=slab[:, 0:4, :], in_=x_r[:, 0:4, :])
                    nc.sync.dma_start(out=slab[:, 4:8, :], in_=x_r[:, 4:8, :])
                    xT = xT_pool.tile([128, 8, S], F32, tag="xT")
                    for half in range(2):
                        for dc in range(8):
                            tr = tr_pool.tile([128, 512], F32, tag="tr")
                            for j in range(4):
                                sc = half * 4 + j
                                nc.tensor.transpose(
                                    tr[:, j * 128:(j + 1) * 128],
                                    slab[:, sc, dc * 128:(dc + 1) * 128],
                                    identity,
                                )
                            # ScalarE is idle during this phase; use it for
                            # the PSUM evacuations to keep DVE free
                            dst = _mm(xT[:, dc, half * 512:(half + 1) * 512])
                            if evac_engine == "scalar":
                                nc.scalar.copy(dst, tr)
                            else:
                                nc.vector.tensor_copy(dst, tr)
                    return xT

                # Q^T and K^T: [hd, seq]
                for name, x_ext, w_ext, bias in (
                    ("wq", xq_e, wq_e, bq_t), ("wk", xk_e, wk_e, bk_t)
                ):
                    w_t = wpool.tile([128, 8, HD], F32, tag="w")
                    nc.gpsimd.dma_start(
                        out=_mm(w_t[:]),
                        in_=w_ext[:].rearrange("(dc p) c -> p dc c", p=128),
                    )
                    xT = transpose_input(x_ext)
                    dstT = qT if name == "wq" else kT
                    for mt in range(4):
                        ps = proj_pool.tile([128, S], F32, tag="proj")
                        for dc in range(8):
                            lhsT = _mm(w_t[:, dc, mt * 128:(mt + 1) * 128])
                            for nq in range(2):
                                nc.tensor.matmul(
                                    ps[:, nq * 512:(nq + 1) * 512],
                                    lhsT,
                                    _mm(xT[:, dc, nq * 512:(nq + 1) * 512]),
                                    start=(dc == 0),
                                    stop=(dc == 7),
                                )
                        nc.vector.tensor_scalar_add(
                            _mm(dstT[:, mt, 0:S]), ps, bias[:, mt:mt + 1]
                        )

                # V: [seq, hd] packed with ones columns
                w_t = wpool.tile([128, 8, HD], F32, tag="w")
                nc.gpsimd.dma_start(
                    out=_mm(w_t[:]),
                    in_=wv_e[:].rearrange("(dc p) c -> p dc c", p=128),
                )
                xT = transpose_input(xv_e)
                for st in range(8):
                    ps = proj_pool.tile([128, HD], F32, tag="projv")
                    for dc in range(8):
                        nc.tensor.matmul(
                            ps,
                            _mm(xT[:, dc, st * 128:(st + 1) * 128]),
                            _mm(w_t[:, dc, :]),
                            start=(dc == 0),
                            stop=(dc == 7),
                        )
                    nc.vector.tensor_add(
                        _mm(vt[:, st, :].rearrange("p (h c) -> p h c", c=66)[:, :, 0:64]),
                        ps[:].rearrange("p (h c) -> p h c", c=64),
                        bv_bc[:].rearrange("p (h c) -> p h c", c=64),
                    )

                # memory-slot columns of K^T: scale_m * mk^T  (no bias)
                for hw in range(4):
                    tr = tr_pool.tile([128, 512], F32, tag="tr")
                    nc.tensor.transpose(
                        tr[:, 0:128], mk_sb[:, hw * 128:(hw + 1) * 128],
                        identity,
                    )
                    nc.vector.tensor_scalar_mul(
                        _mm(kT[:, hw, S:SKM]), tr[:, 0:128], SCALE_M
                    )

            # ---- attention -----------------------------------------------
            run_attn = stop_after != "proj"
            run_wo = run_attn and stop_after != "attn"
            run_rs = run_wo and stop_after != "wo"
            if not run_attn:
                nc.sync.dma_start(out=out_e[:], in_=qT[:])
                run_attn = run_wo = run_rs = False
            sc_width = (2 * S) if es_pair else S
            if run_attn:
                with tc.tile_pool(name="attn_persist", bufs=1) as ap_pool:
                    # attn out^T: rows 0..63 = head dims, row 64 = sums
                    outT = ap_pool.tile([65, NH, S], F32)
                    with tc.tile_pool(name="expS", bufs=es_bufs) as es_pool, \
                         tc.tile_pool(name="bcast", bufs=2) as bc_pool, \
                         tc.tile_pool(name="score_ps", bufs=sc_bufs,
                                      space="PSUM") as sc_pool, \
                         tc.tile_pool(name="av_ps", bufs=2,
                                      space="PSUM") as av_pool:
                        for h in range(NH):
                            hw, hp = h // 2, 64 * (h % 2)
                            outp = av_pool.tile([128, S], F32, tag="av")
                            for grp in kc_groups:
                                width = len(grp) * S
                                sc_ps = sc_pool.tile([128, sc_width], F32,
                                                     tag="sc")
                                for gi, kc in enumerate(grp):
                                    lhsT = _mm(kT[hp:hp + 64, hw,
                                                  kc * 128:(kc + 1) * 128])
                                    for nq in range(2):
                                        col = gi * S + nq * 512
                                        nc.tensor.matmul(
                                            sc_ps[:, col:col + 512],
                                            lhsT,
                                            _mm(qT[hp:hp + 64, hw,
                                                   nq * 512:(nq + 1) * 512]),
                                            start=True, stop=True,
                                        )
                                es = es_pool.tile([128, sc_width], F32,
                                                  tag="es")
                                nc.scalar.activation(
                                    _mm(es[:, 0:width]), sc_ps[:, 0:width],
                                    mybir.ActivationFunctionType.Exp,
                                    scale=INV_SQRT_DK,
                                )
                                for gi, kc in enumerate(grp):
                                    vh = _mm(vt[:, kc, 66 * h:66 * h + 66])
                                    for nq in range(2):
                                        col = gi * S + nq * 512
                                        nc.tensor.matmul(
                                            outp[0:66,
                                                 nq * 512:(nq + 1) * 512],
                                            vh,
                                            _mm(es[:, col:col + 512]),
                                            start=(kc == 0),
                                            stop=(kc == NKC - 1),
                                        )
                            # evacuate out rows + sums row in one copy
                            nc.vector.tensor_copy(_mm(outT[0:65, h, :]),
                                                  outp[0:65, :])

                            # normalize head hh: move sums row to partition 0
                            # (DMA), reciprocal, broadcast via a K=1 ones
                            # matmul, one multiply.  Deferred by one head so
                            # the next head's matmuls outrank it in program
                            # order (avoids stalling PE on the DMA+recip).
                            def normalize(hh):
                                with nc.allow_low_precision(reason="f32r"):
                                    nc.vector.reciprocal(
                                        _mm(outT[64:65, hh, :]),
                                        outT[64:65, hh, :])
                                bc_ps = av_pool.tile([128, S], F32, tag="av")
                                for nq in range(2):
                                    nc.tensor.matmul(
                                        bc_ps[0:64, nq * 512:(nq + 1) * 512],
                                        _mm(ones_t[64:65, 0:64]),
                                        _mm(outT[64:65, hh,
                                                 nq * 512:(nq + 1) * 512]),
                                        start=True, stop=True,
                                    )
                                nc.vector.tensor_mul(_mm(outT[0:64, hh, :]),
                                                     outT[0:64, hh, :],
                                                     bc_ps[0:64, 0:S])
                            if h > 0:
                                normalize(h - 1)
                            if h == NH - 1:
                                normalize(h)

                    # ---- output projection ------------------------------
                    if not run_wo:
                        nc.sync.dma_start(out=out_e[:], in_=outT[0:64, :, :])
                    else:
                        with tc.tile_pool(name="osb", bufs=3) as o_pool, \
                             tc.tile_pool(name="wo_ps", bufs=2,
                                          space="PSUM") as wo_ps_pool:
                            for mt in range(8):
                                ps = wo_ps_pool.tile([128, UNITS], F32,
                                                     tag="wops")
                                for h in range(NH):
                                    lhsT = _mm(outT[0:64, h,
                                                    mt * 128:(mt + 1) * 128])
                                    for nq in range(2):
                                        nc.tensor.matmul(
                                            ps[:, nq * 512:(nq + 1) * 512],
                                            lhsT,
                                            _mm(wo_sb[0:64, h,
                                                      nq * 512:(nq + 1) * 512]),
                                            start=(h == 0),
                                            stop=(h == NH - 1),
                                        )
                                osb = o_pool.tile([128, UNITS], F32,
                                                  tag="osb")
                                # bo comes in already zeroed on odd cores
                                nc.vector.tensor_add(osb, ps, bo_bc)
                                nc.sync.dma_start(
                                    out=partial[mt * 128:(mt + 1) * 128, :],
                                    in_=osb,
                                )

        # ---- pairwise ReduceScatter --------------------------------------
        if run_rs:
            nc.gpsimd.collective_compute(
                "ReduceScatter",
                mybir.AluOpType.add,
                replica_groups=[[0, 1], [2, 3], [4, 5], [6, 7]],
                ins=[partial[:].opt()],
                outs=[rs_out[:].opt()],
            )
            nc.sync.dma_start(out=out_e[:], in_=rs_out[:])
        elif run_wo:
            nc.sync.dma_start(out=out_e[:], in_=partial[0:512, :])

    nc.compile()
    return nc


def _get_nc():
    if "nc" not in _CACHED:
        _CACHED["nc"] = build_nc()
    return _CACHED["nc"]


def _in_maps(queries, keys, values, Wq, bq, Wk, bk, Wv, bv, Wo, bo, mk, mv):
    zeros_bo = np.zeros_like(bo)
    maps = []
    for c in range(8):
        b, g = c // 2, c % 2
        sl = slice(g * HD, (g + 1) * HD)
        maps.append({
            "xq": np.ascontiguousarray(queries[b]),
            "xk": np.ascontiguousarray(keys[b]),
            "xv": np.ascontiguousarray(values[b]),
            "wq": np.ascontiguousarray(Wq[:, sl]),
            "wk": np.ascontiguousarray(Wk[:, sl]),
            "wv": np.ascontiguousarray(Wv[:, sl]),
            "bq": np.ascontiguousarray(bq[sl]),
            "bk": np.ascontiguousarray(bk[sl]),
            "bv": np.ascontiguousarray(bv[sl]),
            "wo": np.ascontiguousarray(Wo[sl, :]),
            "bo": bo if g == 0 else zeros_bo,
            "mk": np.ascontiguousarray(mk[:, sl]),
            "mv": np.ascontiguousarray(mv[:, sl]),
        })
    return maps


def kernel(queries, keys, values, Wq, bq, Wk, bk, Wv, bv, Wo, bo, mk, mv, h=16,
           **_unused):
    queries = np.asarray(queries, np.float32)
    keys = np.asarray(keys, np.float32)
    values = np.asarray(values, np.float32)
    Wq = np.asarray(Wq, np.float32)
    Wk = np.asarray(Wk, np.float32)
    Wv = np.asarray(Wv, np.float32)
    Wo = np.asarray(Wo, np.float32)
    bq = np.asarray(bq, np.float32)
    bk = np.asarray(bk, np.float32)
    bv = np.asarray(bv, np.float32)
    bo = np.asarray(bo, np.float32)
    mk = np.asarray(mk, np.float32).reshape(M, -1)
    mv = np.asarray(mv, np.float32).reshape(M, -1)

    nc = _get_nc()
    in_maps = _in_maps(queries, keys, values, Wq, bq, Wk, bk, Wv, bv, Wo, bo,
                       mk, mv)

    trace = bool(int(os.environ.get("BASS_KERNEL_TRACE", "0")))
    res = run_bass_kernel_spmd(nc, in_maps, list(range(8)), trace=trace)
    _CACHED["last_result"] = res

    out = np.empty((B, S, UNITS), np.float32)
    for c in range(8):
        b, g = c // 2, c % 2
        out[b, g * (S // 2):(g + 1) * (S // 2), :] = res.results[c]["out"]
    return out



# revision 6
# speedup vs baseline: 1.1321x; 1.1321x over previous
"""Multi-head attention with learned memory slots, 8-way sharded for TRN2.

Sharding: 8 cores = 4 batches x 2 head-groups.
  core c -> batch b = c//2, head group g = c%2 (heads 8g..8g+7).
  Wq/Wk/Wv column-sharded by head group, mk/mv sharded on h*d axis,
  Wo row-sharded; pairwise ReduceScatter(add) combines the two head
  groups of a batch and scatters the query rows (4 chunks, overlapped
  with the output projection).

Key points vs the first working version (459us -> target <250us):
  - software-pipelined attention: the score matmuls for key-chunk kc+1
    are emitted BEFORE the AV matmuls for kc, so the PE never stalls
    waiting for exp on the scalar engine (the stall pattern kept the
    HAM throttle at K=4/8 = half clock for the whole attention phase)
  - reciprocal_approx_fast for the softmax denominators (5x faster
    than the iterative DVE reciprocal that burned 52us)
  - output projection contracts head PAIRS (K=128 instead of 64):
    even head's normalized rows go to outP[0:64], odd head's rows are
    DMA-shifted to outP[64:128]; halves the Wo matmul time
  - ReduceScatter in 4 chunks interleaved with the Wo mt-loop, writing
    straight into the output tensor (chunk-interleaved row layout,
    un-permuted on the host)
  - weight/slab DMA ordering + double buffering so the PE never waits
    on input DMA after the first ~6us
"""

import math
import os
from contextlib import ExitStack

import numpy as np

import concourse.bass as bass
import concourse.mybir as mybir
import concourse.tile as tile
from concourse import bacc
from concourse.bass_utils import run_bass_kernel_spmd
from concourse.masks import make_identity

F32 = mybir.dt.float32
MM_DT = mybir.dt.float32r  # matmul operand view; float32r = fast fp32

B = 4
S = 1024          # sequence length (also #queries)
D = 1024          # model dim
NH = 8            # heads per core
DK = 64           # head dim
HD = NH * DK      # 512, per-core head*dim
M = 128           # memory slots
SKM = S + M       # 1152 keys incl. memory slots
NKC = SKM // 128  # 9 key chunks
UNITS = 1024
NPAIR = NH // 2   # head pairs for the output projection
SCALE_M = math.sqrt(float(M))
INV_SQRT_DK = 1.0 / math.sqrt(float(DK))

_CACHED = {}


def _mm(ap):
    return ap.bitcast(MM_DT)


def _bcast_ap(ap, nparts):
    """Partition-broadcast AP: same free pattern on nparts partitions."""
    return bass.AP(tensor=ap.tensor, offset=ap.offset, ap=[[0, nparts]] + list(ap.ap))


def build_nc(rs_chunks=4):
    nc = bacc.Bacc("TRN2", target_bir_lowering=False, debug=False, num_devices=8)

    xq_e = nc.dram_tensor("xq", [S, D], F32, kind="ExternalInput")
    xk_e = nc.dram_tensor("xk", [S, D], F32, kind="ExternalInput")
    xv_e = nc.dram_tensor("xv", [S, D], F32, kind="ExternalInput")
    wq_e = nc.dram_tensor("wq", [D, HD], F32, kind="ExternalInput")
    wk_e = nc.dram_tensor("wk", [D, HD], F32, kind="ExternalInput")
    wv_e = nc.dram_tensor("wv", [D, HD], F32, kind="ExternalInput")
    bq_e = nc.dram_tensor("bq", [HD], F32, kind="ExternalInput")
    bk_e = nc.dram_tensor("bk", [HD], F32, kind="ExternalInput")
    bv_e = nc.dram_tensor("bv", [HD], F32, kind="ExternalInput")
    wo_e = nc.dram_tensor("wo", [HD, UNITS], F32, kind="ExternalInput")
    bo_e = nc.dram_tensor("bo", [UNITS], F32, kind="ExternalInput")
    mk_e = nc.dram_tensor("mk", [M, HD], F32, kind="ExternalInput")
    mv_e = nc.dram_tensor("mv", [M, HD], F32, kind="ExternalInput")
    # chunk-interleaved: row 128*c + i <-> global query row 256*c + 128*g + i
    out_e = nc.dram_tensor("out", [S // 2, UNITS], F32, kind="ExternalOutput")

    with tile.TileContext(nc) as tc, ExitStack() as ctx:
        consts = ctx.enter_context(tc.tile_pool(name="consts", bufs=1))
        dram = ctx.enter_context(tc.tile_pool(name="dram", bufs=1, space="DRAM"))

        identity = consts.tile([128, 128], F32)
        make_identity(nc, identity)

        # biases: bq/bk as [128, 4] per-partition scalars (hd on partitions)
        bq_t = consts.tile([128, 4], F32)
        bk_t = consts.tile([128, 4], F32)
        nc.gpsimd.dma_start(out=bq_t, in_=bq_e[:].rearrange("(mt p) -> p mt", p=128))
        nc.gpsimd.dma_start(out=bk_t, in_=bk_e[:].rearrange("(mt p) -> p mt", p=128))
        # bv/bo broadcast along partitions (they index the free dim)
        bv_bc = consts.tile([128, HD], F32)
        bo_bc = consts.tile([128, UNITS], F32)
        nc.gpsimd.dma_start(out=bv_bc, in_=_bcast_ap(bv_e[:], 128))
        nc.gpsimd.dma_start(out=bo_bc, in_=_bcast_ap(bo_e[:], 128))

        partial = dram.tile([S, UNITS], F32)
        rs_out = dram.tile([S // 2, UNITS], F32)

        with tc.tile_pool(name="qkv", bufs=1) as qkv_pool:
            qT = qkv_pool.tile([128, 4, S], F32)      # [hd_low, hd_grp, q]
            kT = qkv_pool.tile([128, 4, SKM], F32)    # [hd_low, hd_grp, k]
            vt = qkv_pool.tile([128, NKC, NH * 66], F32)  # [k_low, k_chunk, h*66]

            # V layout: head block h = 66 cols: [V_h(64) | ones | pad-ones]
            # (f32r writes/reads need 8B alignment and even element counts;
            #  memset can't emit f32r, so a SWDGE cast-DMA scatters the ones)
            ones_col = consts.tile([128, 2], F32)
            nc.vector.memset(ones_col, 1.0)
            oc = ones_col[:]
            # [1, 64] f32r ones row AT PARTITION 64: lhsT for the K=1
            # recip-broadcast matmuls (matmul lhsT/rhs must share their
            # base partition, and the sums live on partition 64)
            ones_t = consts.tile([65, 64], F32)
            nc.gpsimd.dma_start(
                out=_mm(ones_t[64:65, 0:64]),
                in_=bass.AP(tensor=oc.tensor, offset=oc.offset,
                            ap=[[oc.ap[0][0], 1], [0, 32], [1, 2]]),
            )
            ones_src = bass.AP(
                tensor=oc.tensor, offset=oc.offset,
                ap=[list(oc.ap[0]), [0, NKC * NH], [1, 2]],
            )
            nc.gpsimd.dma_start(
                out=_mm(vt[:].rearrange("p kc (b c) -> p (kc b) c", c=66)[:, :, 64:66]),
                in_=ones_src,
            )

            # mk/mv after the ones (tiny) but before the weights on the
            # gpsimd queue so wq isn't delayed much
            mk_sb = consts.tile([M, HD], F32)
            mv_sb = consts.tile([M, HD], F32)
            nc.gpsimd.dma_start(out=mk_sb, in_=mk_e[:])
            nc.gpsimd.dma_start(out=mv_sb, in_=mv_e[:])

            # memory-slot rows of V (k chunk 8): scale_m * mv, no bias
            nc.vector.tensor_scalar_mul(
                _mm(vt[:, NKC - 1, :].rearrange("p (h c) -> p h c", c=66)[:, :, 0:64]),
                mv_sb[:].rearrange("p (h c) -> p h c", c=64),
                SCALE_M,
            )

            # ---- input transpose + projections ---------------------------
            with tc.tile_pool(name="wproj", bufs=2) as wpool, \
                 tc.tile_pool(name="slab", bufs=2) as slab_pool, \
                 tc.tile_pool(name="xT", bufs=1) as xT_pool, \
                 tc.tile_pool(name="tr_ps", bufs=2, space="PSUM") as tr_pool, \
                 tc.tile_pool(name="proj_ps", bufs=2, space="PSUM") as proj_pool:

                def transpose_input(x_ext):
                    """DRAM [S, D] -> SBUF x^T [128, 8, S] ([d_low, dc, s])."""
                    slab = slab_pool.tile([128, 8, D], F32, tag="slab")
                    # two DMAs: slab[p, sc, c] = x[sc*128 + p, c]; the split
                    # lets the first transposes start at the halfway point
                    x_r = x_ext[:].rearrange("(sc p) c -> p sc c", p=128)
                    nc.sync.dma_start(out=slab[:, 0:4, :], in_=x_r[:, 0:4, :])
                    nc.sync.dma_start(out=slab[:, 4:8, :], in_=x_r[:, 4:8, :])
                    xT = xT_pool.tile([128, 8, S], F32, tag="xT")
                    for half in range(2):
                        for dc in range(8):
                            tr = tr_pool.tile([128, 512], F32, tag="tr")
                            for j in range(4):
                                sc = half * 4 + j
                                nc.tensor.transpose(
                                    tr[:, j * 128:(j + 1) * 128],
                                    slab[:, sc, dc * 128:(dc + 1) * 128],
                                    identity,
                                )
                            # ScalarE is idle during this phase; use it for
                            # the PSUM evacuations to keep DVE free
                            nc.scalar.copy(
                                _mm(xT[:, dc, half * 512:(half + 1) * 512]), tr
                            )
                    return xT

                # Q^T and K^T: [hd, seq]
                for name, x_ext, w_ext, bias in (
                    ("wq", xq_e, wq_e, bq_t), ("wk", xk_e, wk_e, bk_t)
                ):
                    w_t = wpool.tile([128, 8, HD], F32, tag="w")
                    nc.gpsimd.dma_start(
                        out=_mm(w_t[:]),
                        in_=w_ext[:].rearrange("(dc p) c -> p dc c", p=128),
                    )
                    xT = transpose_input(x_ext)
                    dstT = qT if name == "wq" else kT
                    for mt in range(4):
                        ps = proj_pool.tile([128, S], F32, tag="proj")
                        for dc in range(8):
                            lhsT = _mm(w_t[:, dc, mt * 128:(mt + 1) * 128])
                            for nq in range(2):
                                nc.tensor.matmul(
                                    ps[:, nq * 512:(nq + 1) * 512],
                                    lhsT,
                                    _mm(xT[:, dc, nq * 512:(nq + 1) * 512]),
                                    start=(dc == 0),
                                    stop=(dc == 7),
                                )
                        nc.vector.tensor_scalar_add(
                            _mm(dstT[:, mt, 0:S]), ps, bias[:, mt:mt + 1]
                        )

                # V: [seq, hd] packed with ones columns
                w_t = wpool.tile([128, 8, HD], F32, tag="w")
                nc.gpsimd.dma_start(
                    out=_mm(w_t[:]),
                    in_=wv_e[:].rearrange("(dc p) c -> p dc c", p=128),
                )
                xT = transpose_input(xv_e)
                for st in range(8):
                    ps = proj_pool.tile([128, HD], F32, tag="projv")
                    for dc in range(8):
                        nc.tensor.matmul(
                            ps,
                            _mm(xT[:, dc, st * 128:(st + 1) * 128]),
                            _mm(w_t[:, dc, :]),
                            start=(dc == 0),
                            stop=(dc == 7),
                        )
                    nc.vector.tensor_add(
                        _mm(vt[:, st, :].rearrange("p (h c) -> p h c", c=66)[:, :, 0:64]),
                        ps[:].rearrange("p (h c) -> p h c", c=64),
                        bv_bc[:].rearrange("p (h c) -> p h c", c=64),
                    )

                # memory-slot columns of K^T: scale_m * mk^T  (no bias)
                for hw in range(4):
                    tr = tr_pool.tile([128, 512], F32, tag="tr")
                    nc.tensor.transpose(
                        tr[:, 0:128], mk_sb[:, hw * 128:(hw + 1) * 128],
                        identity,
                    )
                    nc.vector.tensor_scalar_mul(
                        _mm(kT[:, hw, S:SKM]), tr[:, 0:128], SCALE_M
                    )

            # Wo loads into SBUF freed by the projection pools; paired-head
            # layout: partition p, pair pp, col c <- Wo[pp*128 + p, c]
            with tc.tile_pool(name="wo_sbuf", bufs=1) as wo_pool, \
                 tc.tile_pool(name="attn_persist", bufs=1) as ap_pool:
                wo_sb = wo_pool.tile([128, NPAIR, UNITS], F32)
                nc.gpsimd.dma_start(
                    out=_mm(wo_sb[:]),
                    in_=wo_e[:].rearrange("(pp p) c -> p pp c", p=128),
                )

                # ---- attention -------------------------------------------
                # per-head attn out^T: rows 0..63 = head dims, row 64 = sums
                outT = ap_pool.tile([65, NH, S], F32)
                # paired layout for the output projection: even head of a
                # pair on partitions 0:64, odd head on 64:128
                outP = ap_pool.tile([128, NPAIR, S], F32)
                with tc.tile_pool(name="expS", bufs=3) as es_pool, \
                     tc.tile_pool(name="score_ps", bufs=2,
                                  space="PSUM") as sc_pool, \
                     tc.tile_pool(name="av_ps", bufs=2,
                                  space="PSUM") as av_pool:

                    def normalize(hh):
                        pp = hh // 2
                        # broadcast the sums row to 64 partitions first, then
                        # reciprocal in place on the PSUM broadcast (the bc
                        # matmul needs an f32r-rounded operand, which the
                        # evac copy produced; recip output feeds only the
                        # DVE multiply, which rounds on its own write)
                        bc = av_pool.tile([128, S], F32, tag="av")
                        for nq in range(2):
                            nc.tensor.matmul(
                                bc[0:64, nq * 512:(nq + 1) * 512],
                                _mm(ones_t[64:65, 0:64]),
                                _mm(outT[64:65, hh, nq * 512:(nq + 1) * 512]),
                                start=True, stop=True,
                            )
                        nc.vector.reciprocal_approx_fast(
                            bc[0:64, 0:S], bc[0:64, 0:S])
                        if hh % 2 == 0:
                            nc.vector.tensor_mul(
                                _mm(outP[0:64, pp, :]),
                                outT[0:64, hh, :], bc[0:64, 0:S])
                        else:
                            nc.vector.tensor_mul(
                                _mm(outT[0:64, hh, :]),
                                outT[0:64, hh, :], bc[0:64, 0:S])
                            # partition shift 0:64 -> 64:128 (DVE is
                            # lane-locked; only DMA can move partitions).
                            # SWDGE cast-DMA keeps the f32r tagging.
                            nc.gpsimd.dma_start(out=_mm(outP[64:128, pp, :]),
                                                in_=outT[0:64, hh, :])

                    for h in range(NH):
                        hw, hp = h // 2, 64 * (h % 2)
                        outp = av_pool.tile([128, S], F32, tag="av")

                        def emit_av(kc, es):
                            vh = _mm(vt[:, kc, 66 * h:66 * h + 66])
                            for nq in range(2):
                                nc.tensor.matmul(
                                    outp[0:66, nq * 512:(nq + 1) * 512],
                                    vh,
                                    _mm(es[:, nq * 512:(nq + 1) * 512]),
                                    start=(kc == 0),
                                    stop=(kc == NKC - 1),
                                    skip_group_check=True,
                                )

                        pend = None
                        for kc in range(NKC):
                            sc_ps = sc_pool.tile([128, S], F32, tag="sc")
                            lhsT = _mm(kT[hp:hp + 64, hw,
                                          kc * 128:(kc + 1) * 128])
                            for nq in range(2):
                                nc.tensor.matmul(
                                    sc_ps[:, nq * 512:(nq + 1) * 512],
                                    lhsT,
                                    _mm(qT[hp:hp + 64, hw,
                                           nq * 512:(nq + 1) * 512]),
                                    start=True, stop=True,
                                )
                            es = es_pool.tile([128, S], F32, tag="es")
                            nc.scalar.activation(
                                _mm(es), sc_ps,
                                mybir.ActivationFunctionType.Exp,
                                scale=INV_SQRT_DK,
                            )
                            # AV for the PREVIOUS key chunk: exp(kc-1) had a
                            # full chunk of PE time to finish, so the PE
                            # never stalls on the scalar engine
                            if pend is not None:
                                emit_av(*pend)
                            pend = (kc, es)
                            # previous head's normalize rides inside this
                            # head's stream (recip on DVE, bc on PE)
                            if kc == 1 and h > 0:
                                normalize(h - 1)
                        emit_av(*pend)
                        # evacuate out rows + sums row in one copy
                        nc.vector.tensor_copy(_mm(outT[0:65, h, :]),
                                              outp[0:65, :])
                        if h == NH - 1:
                            normalize(h)

                # ---- output projection + chunked ReduceScatter ----------
                mt_per_chunk = 8 // rs_chunks
                with tc.tile_pool(name="osb", bufs=3) as o_pool, \
                     tc.tile_pool(name="wo_ps", bufs=2,
                                  space="PSUM") as wo_ps_pool:
                    for mt in range(8):
                        ps = wo_ps_pool.tile([128, UNITS], F32, tag="wops")
                        for pp in range(NPAIR):
                            lhsT = _mm(outP[:, pp, mt * 128:(mt + 1) * 128])
                            for nq in range(2):
                                nc.tensor.matmul(
                                    ps[:, nq * 512:(nq + 1) * 512],
                                    lhsT,
                                    _mm(wo_sb[:, pp, nq * 512:(nq + 1) * 512]),
                                    start=(pp == 0),
                                    stop=(pp == NPAIR - 1),
                                )
                        osb = o_pool.tile([128, UNITS], F32, tag="osb")
                        # bo comes in already zeroed on odd cores
                        nc.vector.tensor_add(osb, ps, bo_bc)
                        nc.sync.dma_start(
                            out=partial[mt * 128:(mt + 1) * 128, :],
                            in_=osb,
                        )
                        if (mt + 1) % mt_per_chunk == 0:
                            c = mt // mt_per_chunk
                            rows = 128 * mt_per_chunk
                            orows = rows // 2
                            nc.gpsimd.collective_compute(
                                "ReduceScatter",
                                mybir.AluOpType.add,
                                replica_groups=[[0, 1], [2, 3], [4, 5], [6, 7]],
                                ins=[partial[c * rows:(c + 1) * rows, :].opt()],
                                outs=[rs_out[c * orows:(c + 1) * orows,
                                             :].opt()],
                            )
                            nc.sync.dma_start(
                                out=out_e[c * orows:(c + 1) * orows, :],
                                in_=rs_out[c * orows:(c + 1) * orows, :],
                            )

    nc.compile()
    return nc


def _get_nc():
    if "nc" not in _CACHED:
        _CACHED["nc"] = build_nc()
    return _CACHED["nc"]


def _in_maps(queries, keys, values, Wq, bq, Wk, bk, Wv, bv, Wo, bo, mk, mv):
    zeros_bo = np.zeros_like(bo)
    maps = []
    for c in range(8):
        b, g = c // 2, c % 2
        sl = slice(g * HD, (g + 1) * HD)
        maps.append({
            "xq": np.ascontiguousarray(queries[b]),
            "xk": np.ascontiguousarray(keys[b]),
            "xv": np.ascontiguousarray(values[b]),
            "wq": np.ascontiguousarray(Wq[:, sl]),
            "wk": np.ascontiguousarray(Wk[:, sl]),
            "wv": np.ascontiguousarray(Wv[:, sl]),
            "bq": np.ascontiguousarray(bq[sl]),
            "bk": np.ascontiguousarray(bk[sl]),
            "bv": np.ascontiguousarray(bv[sl]),
            "wo": np.ascontiguousarray(Wo[sl, :]),
            "bo": bo if g == 0 else zeros_bo,
            "mk": np.ascontiguousarray(mk[:, sl]),
            "mv": np.ascontiguousarray(mv[:, sl]),
        })
    return maps


def kernel(queries, keys, values, Wq, bq, Wk, bk, Wv, bv, Wo, bo, mk, mv, h=16,
           **_unused):
    queries = np.asarray(queries, np.float32)
    keys = np.asarray(keys, np.float32)
    values = np.asarray(values, np.float32)
    Wq = np.asarray(Wq, np.float32)
    Wk = np.asarray(Wk, np.float32)
    Wv = np.asarray(Wv, np.float32)
    Wo = np.asarray(Wo, np.float32)
    bq = np.asarray(bq, np.float32)
    bk = np.asarray(bk, np.float32)
    bv = np.asarray(bv, np.float32)
    bo = np.asarray(bo, np.float32)
    mk = np.asarray(mk, np.float32).reshape(M, -1)
    mv = np.asarray(mv, np.float32).reshape(M, -1)

    nc = _get_nc()
    in_maps = _in_maps(queries, keys, values, Wq, bq, Wk, bk, Wv, bv, Wo, bo,
                       mk, mv)

    trace = bool(int(os.environ.get("BASS_KERNEL_TRACE", "0")))
    res = run_bass_kernel_spmd(nc, in_maps, list(range(8)), trace=trace)
    _CACHED["last_result"] = res

    # out rows are chunk-interleaved: core row 128*c + i is global query
    # row 256*c + 128*g + i of batch b
    out = np.empty((B, S, UNITS), np.float32)
    for core in range(8):
        b, g = core // 2, core % 2
        r = res.results[core]["out"]
        for c in range(4):
            out[b, 256 * c + 128 * g:256 * c + 128 * (g + 1), :] = \
                r[128 * c:128 * (c + 1)]
    return out
